# revision 1
# baseline (speedup 1.0000x reference)
"""Trainium2 Bass kernel for nn_ComplexityAttention (GQA attention block).

Computation (B=1, S=2048, HID=2048, 16 Q heads / 4 KV heads, D=128):
  q/k/v = x @ W^T + mu @ Wm^T           (fused mu-guided projections)
  per-head RMSNorm on q, k; RoPE; causal GQA attention; out @ wo^T.

Sharding: tensor-parallel over heads across 8 NeuronCores. Core c owns
Q heads {2c, 2c+1} and KV head c//2 (KV work duplicated per core pair).
Each core produces a partial output (its heads' slice of wo applied),
host sums the 8 partials.

Device-side layout strategy:
  - Host pre-transposes x/mu to [HID, S] and weights to [HID, out] so all
    matmuls contract over the partition dim with no on-device transposes
    for the projections.
  - Projections computed in [s, d] tiles (one PSUM bank holds q0|q1|k|v),
    RMSNorm+RoPE done with per-partition scalars + free-dim shifts
    (fused scalar_tensor_tensor ops), then Q/K PE-transposed to [d, s]
    for attention.
  - Scores computed transposed: S^T[kv, q] = K^T.T @ Q^T. Softmax without
    max-subtraction (scores bounded by +/-sqrt(128) after RMSNorm, exp is
    safe in fp32); denominator via ones-vector matmul; causal masking via
    4 static multiplicative masks on the diagonal tiles.
  - PV: out^T[d, q] = V[kv, d].T @ expS^T[kv, q] accumulated over kv chunks.
  - Output projection from out^T directly; partial written as [o, s] fp32.

All matmul inputs are bf16 (fp32 PSUM accumulation); statistics in fp32.
"""

import sys

for _p in ("/opt/trn_rl_repo", "/root/.axon_site/_ro/trn_rl_repo"):
    if _p not in sys.path:
        sys.path.insert(0, _p)

import numpy as np
import ml_dtypes

import concourse.bass as bass
import concourse.bacc as bacc
import concourse.mybir as mybir
import concourse.tile as tile
from concourse.bass_utils import run_bass_kernel_spmd
from concourse.masks import make_identity

# Problem constants (hardcoded per contract)
B, S, HID = 1, 2048, 2048
NUM_HEADS, NUM_KV_HEADS, HEAD_DIM = 16, 4, 128
ROPE_THETA = 10000.0
EPS = 1e-6
N_CORES = 8

P = 128
KC = HID // P            # 16 contraction chunks
SC = S // P              # 16 sequence chunks of 128
QCH = 512                # attention q-chunk (one PSUM bank)
NQC = S // QCH           # 4
NPASS = 8                # projection passes (2 s-chunks each)
SCP = SC // NPASS        # s-chunks per pass = 2
QK_SCALE = 1.0 / float(np.sqrt(HEAD_DIM))

BF16 = mybir.dt.bfloat16
F32 = mybir.dt.float32
NP_BF16 = ml_dtypes.bfloat16

_PROGRAM = {}  # repeats -> compiled Bacc program


def _build_program(repeats=1):
    """Build the per-core Bass/Tile program (identical on all 8 cores)."""
    AF = mybir.ActivationFunctionType
    OP = mybir.AluOpType

    nc = bacc.Bacc(trn_type="TRN2", debug=False)

    # ---- DRAM I/O ----
    xT = nc.dram_tensor("xT", [KC, P, S], BF16, kind="ExternalInput")
    muT = nc.dram_tensor("muT", [KC, P, S], BF16, kind="ExternalInput")
    # packed projection weights: [q0 | q1 | k | v] columns, transposed to [HID, 512]
    w_all = nc.dram_tensor("w_all", [KC, P, 512], BF16, kind="ExternalInput")
    wm_all = nc.dram_tensor("wm_all", [KC, P, 512], BF16, kind="ExternalInput")
    woT = nc.dram_tensor("woT", [2, P, HID], BF16, kind="ExternalInput")
    cosq = nc.dram_tensor("cosq", [SC, P, HEAD_DIM], F32, kind="ExternalInput")
    sinq = nc.dram_tensor("sinq", [SC, P, HEAD_DIM], F32, kind="ExternalInput")
    cosk = nc.dram_tensor("cosk", [SC, P, HEAD_DIM], F32, kind="ExternalInput")
    sink = nc.dram_tensor("sink", [SC, P, HEAD_DIM], F32, kind="ExternalInput")
    out_d = nc.dram_tensor("out", [KC, P, S], F32, kind="ExternalOutput")

    with tile.TileContext(nc) as tc:
        with (
            tc.tile_pool(name="persist", bufs=1) as persist,
            tc.tile_pool(name="stream", bufs=8) as stream,
            tc.tile_pool(name="tmp", bufs=6) as tmp,
            tc.tile_pool(name="small", bufs=6) as small,
            tc.tile_pool(name="expp", bufs=6) as expp,
            tc.tile_pool(name="ostage", bufs=6) as ostage,
            tc.tile_pool(name="ps_big", bufs=6, space="PSUM") as ps_big,
            tc.tile_pool(name="ps_scr", bufs=2, space="PSUM") as ps_scr,
        ):
            # ---- persistent SBUF tensors ----
            w_sb = persist.tile([P, KC, 512], BF16, name="w_sb")
            wm_sb = persist.tile([P, KC, 512], BF16, name="wm_sb")
            wo_sb = persist.tile([P, 2, HID], BF16, name="wo_sb")
            cq_sb = persist.tile([P, SC, HEAD_DIM], F32, name="cq_sb")
            sq_sb = persist.tile([P, SC, HEAD_DIM], F32, name="sq_sb")
            ck_sb = persist.tile([P, SC, HEAD_DIM], F32, name="ck_sb")
            sk_sb = persist.tile([P, SC, HEAD_DIM], F32, name="sk_sb")
            qt_sb = [
                persist.tile([P, S], BF16, name=f"qt{h}_sb") for h in range(2)
            ]
            kt_sb = persist.tile([P, S], BF16, name="kt_sb")
            v_sb = persist.tile([P, SC, HEAD_DIM], BF16, name="v_sb")
            attn_sb = [
                persist.tile([P, S], BF16, name=f"attn{c}_sb") for c in range(2)
            ]
            ident = persist.tile([P, P], BF16, name="ident")
            ones_sb = persist.tile([P, 1], BF16, name="ones_sb")
            eps_sb = persist.tile([P, 1], F32, name="eps_sb")
            masks = [
                persist.tile([P, P], BF16, name=f"mask{r}") for r in range(1)
            ]

            make_identity(nc, ident[:])
            nc.gpsimd.memset(ones_sb[:], 1.0)
            nc.gpsimd.memset(eps_sb[:], EPS)
            for r in range(1):
                # keep 1.0 where (q_local - kv_local) >= 0, else 0
                nc.gpsimd.memset(masks[r][:], 1.0)
                nc.gpsimd.affine_select(
                    out=masks[r][:],
                    in_=masks[r][:],
                    compare_op=mybir.AluOpType.is_ge,
                    fill=0.0,
                    base=0,
                    pattern=[[1, P]],
                    channel_multiplier=-1,
                )

            # head offsets inside the packed 512-wide projection output
            # (q0, q1, k occupy 0:128, 128:256, 256:384 and get norm+rope;
            #  v occupies 384:512)
            norm_specs = [
                (2, ck_sb, sk_sb, kt_sb),
                (0, cq_sb, sq_sb, qt_sb[0]),
                (1, cq_sb, sq_sb, qt_sb[1]),
            ]

            def attention_scores_pv(qc):
                """scores/exp/PV/den accumulation for q chunk qc; returns
                (out_ps, den_ps) per head."""
                jpq = QCH // P  # kv chunks per q chunk
                jmax = jpq * qc + (jpq - 1)
                q_sl = slice(qc * QCH, (qc + 1) * QCH)
                out_ps = [
                    ps_big.tile([P, QCH], F32, tag="big", name=f"out_ps{h}")
                    for h in range(2)
                ]
                den_ps = [
                    ps_scr.tile([1, QCH], F32, tag="scr", name=f"den_ps{h}")
                    for h in range(2)
                ]
                for j in range(jmax + 1):
                    r = j - jpq * qc
                    for h in range(2):
                        s_ps = ps_big.tile([P, QCH], F32, tag="big", name="s_ps")
                        nc.tensor.matmul(
                            s_ps[:],
                            kt_sb[:, j * P : (j + 1) * P],
                            qt_sb[h][:, q_sl],
                            start=True,
                            stop=True,
                        )
                        e = expp.tile([P, QCH], BF16, tag="e", name="e")
                        if r > 0:
                            # columns < 128*r are fully masked: zero them and
                            # exp only the live tail
                            nc.vector.memset(e[:, : P * r], 0.0)
                            nc.scalar.activation(
                                e[:, P * r :], s_ps[:, P * r :], AF.Exp,
                                scale=QK_SCALE,
                            )
                        else:
                            nc.scalar.activation(
                                e[:], s_ps[:], AF.Exp, scale=QK_SCALE
                            )
                        if r >= 0:
                            # triangular mask on the 128-wide diagonal block
                            nc.vector.tensor_mul(
                                e[:, P * r : P * (r + 1)],
                                e[:, P * r : P * (r + 1)],
                                masks[0][:],
                            )
                        nc.tensor.matmul(
                            out_ps[h][:],
                            v_sb[:, j, :],
                            e[:],
                            start=(j == 0),
                            stop=(j == jmax),
                        )
                        nc.tensor.matmul(
                            den_ps[h][:],
                            ones_sb[:],
                            e[:],
                            start=(j == 0),
                            stop=(j == jmax),
                        )
                return out_ps, den_ps

            def attention_div(qc, out_ps, den_ps):
                q_sl = slice(qc * QCH, (qc + 1) * QCH)
                for h in range(2):
                    rd = small.tile([1, QCH], F32, tag="rd", name="rd")
                    nc.vector.reciprocal(rd[:], den_ps[h][:])
                    rdb = tmp.tile([P, QCH], F32, tag="rdb", name="rdb")
                    nc.gpsimd.partition_broadcast(rdb[:], rd[:])
                    nc.vector.tensor_mul(
                        attn_sb[h][:, q_sl], out_ps[h][:], rdb[:]
                    )

            def do_wo(qc):
                """output projection for q chunk qc: out_pT[o, q] partial."""
                q_sl = slice(qc * QCH, (qc + 1) * QCH)
                for oc in range(KC):
                    o_ps = ps_big.tile([P, QCH], F32, tag="big", name="o_ps")
                    for c in range(2):
                        nc.tensor.matmul(
                            o_ps[:],
                            wo_sb[:, c, oc * P : (oc + 1) * P],
                            attn_sb[c][:, q_sl],
                            start=(c == 0),
                            stop=(c == 1),
                        )
                    ob = ostage.tile([P, QCH], F32, tag="ob", name="ob")
                    nc.vector.tensor_copy(ob[:], o_ps[:])
                    nc.scalar.dma_start(
                        out_d.ap()[oc, :, q_sl], ob[:]
                    )

            for rep in range(repeats):
                for p in range(NPASS):
                    col0 = p * SCP * P  # first s column of this pass (512 wide)
                    psums = [
                        ps_big.tile([P, 512], F32, tag="big", name=f"proj{p}_{i}")
                        for i in range(SCP)
                    ]
                    # x @ W^T contributions
                    for kc in range(KC):
                        if p == 0 and rep == 0:
                            nc.scalar.dma_start(w_sb[:, kc, :], w_all.ap()[kc])
                        xt = stream.tile([P, SCP * P], BF16, tag="xt", name="xt")
                        nc.sync.dma_start(xt[:], xT.ap()[kc, :, col0 : col0 + SCP * P])
                        for i in range(SCP):
                            nc.tensor.matmul(
                                psums[i][:],
                                xt[:, i * P : (i + 1) * P],
                                w_sb[:, kc, :],
                                start=(kc == 0),
                                stop=False,
                            )
                    # mu @ Wm^T contributions
                    for kc in range(KC):
                        if p == 0 and rep == 0:
                            nc.scalar.dma_start(wm_sb[:, kc, :], wm_all.ap()[kc])
                        mt = stream.tile([P, SCP * P], BF16, tag="mt", name="mt")
                        nc.sync.dma_start(mt[:], muT.ap()[kc, :, col0 : col0 + SCP * P])
                        for i in range(SCP):
                            nc.tensor.matmul(
                                psums[i][:],
                                mt[:, i * P : (i + 1) * P],
                                wm_sb[:, kc, :],
                                start=False,
                                stop=(kc == KC - 1),
                            )
                    if p == 0 and rep == 0:
                        for sc4 in range(0, SC, 4):
                            nc.scalar.dma_start(
                                cq_sb[:, sc4 : sc4 + 4, :],
                                cosq.ap()[sc4 : sc4 + 4].rearrange("s p d -> p s d"),
                            )
                            nc.scalar.dma_start(
                                sq_sb[:, sc4 : sc4 + 4, :],
                                sinq.ap()[sc4 : sc4 + 4].rearrange("s p d -> p s d"),
                            )
                            nc.scalar.dma_start(
                                ck_sb[:, sc4 : sc4 + 4, :],
                                cosk.ap()[sc4 : sc4 + 4].rearrange("s p d -> p s d"),
                            )
                            nc.scalar.dma_start(
                                sk_sb[:, sc4 : sc4 + 4, :],
                                sink.ap()[sc4 : sc4 + 4].rearrange("s p d -> p s d"),
                            )
                        for c in range(2):
                            nc.scalar.dma_start(wo_sb[:, c, :], woT.ap()[c])
                    # RMSNorm + RoPE + transpose to [d, s]; V copy
                    for i in range(SCP):
                        sc = p * SCP + i
                        ps = psums[i]
                        for hidx, c_sb, s_sb, dst in norm_specs:
                            off = hidx * P
                            sqv = tmp.tile([P, HEAD_DIM], F32, tag="sqv", name="sqv")
                            var = small.tile([P, 1], F32, tag="var", name="var")
                            nc.scalar.activation(
                                sqv[:], ps[:, off : off + P], AF.Square, accum_out=var[:]
                            )
                            std = small.tile([P, 1], F32, tag="std", name="std")
                            # std = sqrt(sum(q^2)/D + eps)
                            nc.scalar.activation(
                                std[:], var[:], AF.Sqrt, scale=1.0 / HEAD_DIM, bias=eps_sb[:]
                            )
                            rstd = small.tile([P, 1], F32, tag="rstd", name="rstd")
                            nc.vector.reciprocal(rstd[:], std[:])
                            t1 = tmp.tile([P, HEAD_DIM], F32, tag="t1", name="t1")
                            nc.vector.scalar_tensor_tensor(
                                t1[:],
                                ps[:, off : off + P],
                                rstd[:],
                                c_sb[:, sc, :],
                                op0=OP.mult,
                                op1=OP.mult,
                            )
                            t2 = tmp.tile([P, HEAD_DIM], F32, tag="t2", name="t2")
                            nc.vector.scalar_tensor_tensor(
                                t2[:, 0:64],
                                ps[:, off + 64 : off + P],
                                rstd[:],
                                s_sb[:, sc, 0:64],
                                op0=OP.mult,
                                op1=OP.mult,
                            )
                            nc.vector.scalar_tensor_tensor(
                                t2[:, 64:P],
                                ps[:, off : off + 64],
                                rstd[:],
                                s_sb[:, sc, 64:P],
                                op0=OP.mult,
                                op1=OP.mult,
                            )
                            qsd = tmp.tile([P, HEAD_DIM], BF16, tag="qsd", name="qsd")
                            nc.vector.tensor_add(qsd[:], t1[:], t2[:])
                            tr = ps_scr.tile([P, P], BF16, tag="scr", name="tr")
                            nc.tensor.transpose(tr[:], qsd[:], ident[:])
                            nc.vector.tensor_copy(
                                dst[:, sc * P : (sc + 1) * P], tr[:]
                            )
                        # V: plain copy (cast) into [s, d] layout
                        nc.scalar.copy(v_sb[:, sc, :], ps[:, 384:512])
                # attention + output projection, after all projections
                # (keeps ACT on one table set per phase: sqrt/square first, exp after)
                for qc in range(NQC):
                    acc = attention_scores_pv(qc)
                    if qc > 0:
                        do_wo(qc - 1)
                    attention_div(qc, *acc)
                do_wo(NQC - 1)


    nc.compile()
    return nc


def _get_program(repeats=1):
    if repeats not in _PROGRAM:
        _PROGRAM[repeats] = _build_program(repeats)
    return _PROGRAM[repeats]


def _host_prepare(inputs):
    """Shard + lay out inputs for the 8 cores."""
    hs = np.asarray(inputs["hidden_states"], dtype=np.float32).reshape(S, HID)
    mu = np.asarray(inputs["mu_prev"], dtype=np.float32).reshape(S, HID)
    wq = np.asarray(inputs["wq"], dtype=np.float32)
    wk = np.asarray(inputs["wk"], dtype=np.float32)
    wv = np.asarray(inputs["wv"], dtype=np.float32)
    wo = np.asarray(inputs["wo"], dtype=np.float32)
    wmq = np.asarray(inputs["wmq"], dtype=np.float32)
    wmk = np.asarray(inputs["wmk"], dtype=np.float32)
    wmv = np.asarray(inputs["wmv"], dtype=np.float32)
    qw = np.asarray(inputs["q_norm_w"], dtype=np.float32)
    kw = np.asarray(inputs["k_norm_w"], dtype=np.float32)

    xT = np.ascontiguousarray(hs.T).astype(NP_BF16).reshape(KC, P, S)
    muT = np.ascontiguousarray(mu.T).astype(NP_BF16).reshape(KC, P, S)

    # RoPE tables in [s, d] layout with rotate-half sign and norm weight baked in
    inv = 1.0 / (ROPE_THETA ** (np.arange(0, HEAD_DIM, 2, dtype=np.float32) / HEAD_DIM))
    ang = np.arange(S, dtype=np.float32)[:, None] * inv[None, :]  # [S, 64]
    emb = np.concatenate([ang, ang], axis=-1)  # [S, 128]
    cos_e = np.cos(emb)
    sin_e = np.sin(emb)
    sin_s = np.concatenate([-sin_e[:, :64], sin_e[:, 64:]], axis=-1)

    def tables(w):
        w_shift = np.concatenate([w[64:], w[:64]])
        cos_t = (cos_e * w[None, :]).astype(np.float32).reshape(SC, P, HEAD_DIM)
        sin_t = (sin_s * w_shift[None, :]).astype(np.float32).reshape(SC, P, HEAD_DIM)
        return np.ascontiguousarray(cos_t), np.ascontiguousarray(sin_t)

    cq, sq = tables(qw)
    ck, sk = tables(kw)

    in_maps = []
    for c in range(N_CORES):
        g = c // 2
        wq_s = wq[256 * c : 256 * (c + 1)]      # [256, HID]
        wmq_s = wmq[256 * c : 256 * (c + 1)]
        wk_s = wk[P * g : P * (g + 1)]          # [128, HID]
        wmk_s = wmk[P * g : P * (g + 1)]
        wv_s = wv[P * g : P * (g + 1)]
        wmv_s = wmv[P * g : P * (g + 1)]
        w_all = np.concatenate([wq_s.T, wk_s.T, wv_s.T], axis=1)     # [HID, 512]
        wm_all = np.concatenate([wmq_s.T, wmk_s.T, wmv_s.T], axis=1)
        woT_c = wo[:, 256 * c : 256 * (c + 1)].T                     # [256, HID]
        in_maps.append(
            {
                "xT": xT,
                "muT": muT,
                "w_all": np.ascontiguousarray(w_all).astype(NP_BF16).reshape(KC, P, 512),
                "wm_all": np.ascontiguousarray(wm_all).astype(NP_BF16).reshape(KC, P, 512),
                "woT": np.ascontiguousarray(woT_c).astype(NP_BF16).reshape(2, P, HID),
                "cosq": cq,
                "sinq": sq,
                "cosk": ck,
                "sink": sk,
            }
        )
    return in_maps


def run(inputs, trace=False):
    """Run the SPMD kernel; returns (full_output, exec_time_ns_or_None)."""
    nc = _get_program()
    in_maps = _host_prepare(inputs)
    res = run_bass_kernel_spmd(
        nc, in_maps, core_ids=list(range(N_CORES)), trace=trace
    )
    total = np.zeros((HID, S), dtype=np.float32)
    for c in range(N_CORES):
        total += res.results[c]["out"].reshape(HID, S)
    out = np.ascontiguousarray(total.T).reshape(B, S, HID).astype(np.float32)
    return out, res.exec_time_ns


def kernel(**inputs) -> np.ndarray:
    out, _ = run(inputs, trace=False)
    return out



# revision 11
# speedup vs baseline: 1.4419x; 1.4419x over previous
"""Trainium2 Bass kernel for nn_ComplexityAttention (GQA attention block).

Computation (B=1, S=2048, HID=2048, 16 Q heads / 4 KV heads, D=128):
  q/k/v = x @ W^T + mu @ Wm^T           (fused mu-guided projections)
  per-head RMSNorm on q, k; RoPE; causal GQA attention; out @ wo^T.

Sharding: tensor-parallel over heads across 8 NeuronCores. Core c owns
Q heads {2c, 2c+1} and KV head c//2 (KV work duplicated per core pair).
Each core produces a partial output (its heads' slice of wo applied),
host sums the 8 partials.

Device-side layout strategy:
  - Host pre-transposes x/mu to [HID, S] and packs them into per-pass
    blocks laid out contiguously per SBUF partition, so each projection
    pass streams with ONE large DMA (the shared HWDGE descriptor-gen
    device costs ~630ns per DMA instruction; many small DMAs starve PE).
  - Projections computed in [s, d] tiles (one PSUM bank holds q0|q1|k|v),
    RMSNorm+RoPE done with per-partition scalars + free-dim shifts
    (fused scalar_tensor_tensor ops), then Q/K PE-transposed to [d, s]
    for attention.
  - Scores computed transposed: S^T[kv, q] = K^T.T @ Q^T. Softmax without
    max-subtraction (scores bounded by +/-sqrt(128) after RMSNorm, exp is
    safe in fp32); denominator via ones-vector matmul; causal masking via
    a static multiplicative mask on the diagonal tiles. Fully-masked
    column ranges of partially-causal diagonal blocks are trimmed from
    the scores/exp/PV/den ops.
  - PV: out^T[d, q] = V[kv, d].T @ expS^T[kv, q] accumulated over kv chunks.
  - Output projection from out^T directly; partials staged in SBUF as
    bf16 and written with one DMA per 512-wide q chunk.

All matmul inputs are bf16 (fp32 PSUM accumulation); statistics in fp32.
"""

import sys

for _p in ("/opt/trn_rl_repo", "/root/.axon_site/_ro/trn_rl_repo"):
    if _p not in sys.path:
        sys.path.insert(0, _p)

import numpy as np
import ml_dtypes

import concourse.bass as bass
import concourse.bacc as bacc
import concourse.mybir as mybir
import concourse.hw_specs as hw_specs
import concourse.tile as tile
from concourse.bass_utils import run_bass_kernel_spmd
from concourse.masks import make_identity

# Problem constants (hardcoded per contract)
B, S, HID = 1, 2048, 2048
NUM_HEADS, NUM_KV_HEADS, HEAD_DIM = 16, 4, 128
ROPE_THETA = 10000.0
EPS = 1e-6
N_CORES = 8

P = 128
KC = HID // P            # 16 contraction chunks
SC = S // P              # 16 sequence chunks of 128
QCH = 512                # attention q-chunk (one PSUM bank)
NQC = S // QCH           # 4
NPASS = 8                # projection passes (2 s-chunks each)
SCP = SC // NPASS        # s-chunks per pass = 2
PASS_W = SCP * P         # 256 s columns per pass
WG = 8                   # weight-load groups (KC/WG kc chunks each)
QK_SCALE = 1.0 / float(np.sqrt(HEAD_DIM))

BF16 = mybir.dt.bfloat16
F32 = mybir.dt.float32
NP_BF16 = ml_dtypes.bfloat16

_PROGRAM = {}  # repeats -> compiled Bacc program

_ONE_SET = "natural_log_exp_and_others"  # act_info set 6: square+ln+exp+copy


def _pin_act_tables():
    """Make every activation func this kernel uses resolve ONLY to the
    natural_log_exp_and_others table set. The set-placement pass otherwise
    assigns Square/Exp to one set and Ln to another, inserting a 1283ns
    table reload between nearly every pair of activations."""
    if getattr(bacc, "_act_tables_pinned", False):
        return
    AF = mybir.ActivationFunctionType
    ours = {AF.Square, AF.Ln, AF.Exp, AF.Copy, AF.Identity}
    real_get = hw_specs.get_activation_tables

    def patched(arch):
        tabs = real_get(arch)
        return {
            name: (s if name == _ONE_SET else s - ours)
            for name, s in tabs.items()
        }

    bacc.get_activation_tables = patched
    bacc._act_tables_pinned = True


def _build_program(repeats=1):
    """Build the per-core Bass/Tile program (identical on all 8 cores)."""
    AF = mybir.ActivationFunctionType
    OP = mybir.AluOpType

    _pin_act_tables()
    nc = bacc.Bacc(trn_type="TRN2", debug=False)

    # ---- DRAM I/O (all pre-packed on host for single-DMA loads) ----
    xP = nc.dram_tensor("xP", [NPASS, P, KC * PASS_W], BF16, kind="ExternalInput")
    muP = nc.dram_tensor("muP", [NPASS, P, KC * PASS_W], BF16, kind="ExternalInput")
    # packed projection weights: [q0 | q1 | k | v] columns, [P, kc, 512] layout
    wP = nc.dram_tensor("wP", [WG, P, KC // WG, 512], BF16, kind="ExternalInput")
    wmP = nc.dram_tensor("wmP", [WG, P, KC // WG, 512], BF16, kind="ExternalInput")
    woP = nc.dram_tensor("woP", [2, P, HID], BF16, kind="ExternalInput")
    cosq = nc.dram_tensor("cosq", [1, P, SC, HEAD_DIM], BF16, kind="ExternalInput")
    sinq = nc.dram_tensor("sinq", [1, P, SC, HEAD_DIM], BF16, kind="ExternalInput")
    cosk = nc.dram_tensor("cosk", [1, P, SC, HEAD_DIM], BF16, kind="ExternalInput")
    sink = nc.dram_tensor("sink", [1, P, SC, HEAD_DIM], BF16, kind="ExternalInput")
    out_d = nc.dram_tensor("out", [NQC, P, KC * QCH], BF16, kind="ExternalOutput")

    KCG = KC // WG  # kc chunks per weight-load group

    with tile.TileContext(nc) as tc:
        with (
            tc.tile_pool(name="persist", bufs=1) as persist,
            tc.tile_pool(name="stream", bufs=3) as stream,
            tc.tile_pool(name="tmp", bufs=4) as tmp,
            tc.tile_pool(name="small", bufs=4) as small,
            tc.tile_pool(name="expp", bufs=6) as expp,
            tc.tile_pool(name="ostage", bufs=2) as ostage,
            tc.tile_pool(name="rstp", bufs=14) as rstp,
            tc.tile_pool(name="ps_att", bufs=4, space="PSUM") as ps_att,
            tc.tile_pool(name="ps_wo", bufs=2, space="PSUM") as ps_wo,
            tc.tile_pool(name="ps_scr", bufs=2, space="PSUM") as ps_scr,
        ):
            # ---- persistent SBUF tensors ----
            w_sb = persist.tile([P, KC, 512], BF16, name="w_sb")
            wm_sb = persist.tile([P, KC, 512], BF16, name="wm_sb")
            wo_sb = persist.tile([P, 2, HID], BF16, name="wo_sb")
            cq_sb = persist.tile([P, SC, HEAD_DIM], BF16, name="cq_sb")
            sq_sb = persist.tile([P, SC, HEAD_DIM], BF16, name="sq_sb")
            ck_sb = persist.tile([P, SC, HEAD_DIM], BF16, name="ck_sb")
            sk_sb = persist.tile([P, SC, HEAD_DIM], BF16, name="sk_sb")
            qt_sb = [
                persist.tile([P, S], BF16, name=f"qt{h}_sb") for h in range(2)
            ]
            kt_sb = persist.tile([P, S], BF16, name="kt_sb")
            v_sb = persist.tile([P, SC, HEAD_DIM], BF16, name="v_sb")
            attn_sb = [
                persist.tile([P, S], BF16, name=f"attn{c}_sb") for c in range(2)
            ]
            ident = persist.tile([P, P], BF16, name="ident")
            ones_sb = persist.tile([P, 1], BF16, name="ones_sb")
            eps_sb = persist.tile([P, 1], F32, name="eps_sb")
            diag_mask = persist.tile([P, P], BF16, name="diag_mask")

            make_identity(nc, ident[:])
            nc.gpsimd.memset(ones_sb[:], 1.0)
            nc.gpsimd.memset(eps_sb[:], EPS)
            # keep 1.0 where (q_local - kv_local) >= 0, else 0
            nc.gpsimd.memset(diag_mask[:], 1.0)
            nc.gpsimd.affine_select(
                out=diag_mask[:],
                in_=diag_mask[:],
                compare_op=mybir.AluOpType.is_ge,
                fill=0.0,
                base=0,
                pattern=[[1, P]],
                channel_multiplier=-1,
            )

            # head offsets inside the packed 512-wide projection output
            # (q0, q1, k occupy 0:128, 128:256, 256:384 and get norm+rope;
            #  v occupies 384:512)
            norm_specs = [
                (2, ck_sb, sk_sb, kt_sb),
                (0, cq_sb, sq_sb, qt_sb[0]),
                (1, cq_sb, sq_sb, qt_sb[1]),
            ]

            def attention_head(qc, h):
                """scores/exp/PV/den for one head of q chunk qc; returns
                (out_ps, den_ps). Heads are processed serially so the first
                head's PSUM frees mid-chunk and its softmax-div latency
                hides under the second head's matmuls."""
                jpq = QCH // P  # kv chunks per q chunk
                jmax = jpq * qc + (jpq - 1)
                out_ps = ps_att.tile([P, QCH], F32, tag="att", name=f"out_ps{h}")
                acc = tmp.tile([P, QCH], BF16, tag=f"acc{h}", name=f"acc{h}")
                for j in range(jmax + 1):
                    r = j - jpq * qc
                    # columns < P*r of this block are fully causal-masked.
                    # Trim them from scores/exp/PV/den, except on the last
                    # (stop) block where PV must cover the full PSUM
                    # width: there the dead columns of e are zeroed instead.
                    c0 = P * r if 0 < r < jpq - 1 else 0
                    s_ps = ps_att.tile([P, QCH], F32, tag="att", name="s_ps")
                    e = expp.tile([P, QCH], BF16, tag="e", name="e")
                    if r == jpq - 1:
                        nc.tensor.matmul(
                            s_ps[:, P * r :],
                            kt_sb[:, j * P : (j + 1) * P],
                            qt_sb[h][:, qc * QCH + P * r : (qc + 1) * QCH],
                            start=True,
                            stop=True,
                        )
                        nc.vector.memset(e[:, : P * r], 0.0)
                        nc.scalar.activation(
                            e[:, P * r :], s_ps[:, P * r :], AF.Exp,
                            scale=QK_SCALE,
                        )
                    else:
                        nc.tensor.matmul(
                            s_ps[:, c0:],
                            kt_sb[:, j * P : (j + 1) * P],
                            qt_sb[h][:, qc * QCH + c0 : (qc + 1) * QCH],
                            start=True,
                            stop=True,
                        )
                        nc.scalar.activation(
                            e[:, c0:], s_ps[:, c0:], AF.Exp, scale=QK_SCALE
                        )
                    if r >= 0:
                        # triangular mask on the 128-wide diagonal block
                        nc.vector.tensor_mul(
                            e[:, P * r : P * (r + 1)],
                            e[:, P * r : P * (r + 1)],
                            diag_mask[:],
                        )
                    pv_c0 = c0 if j != jmax else 0
                    nc.tensor.matmul(
                        out_ps[:, pv_c0:],
                        v_sb[:, j, :],
                        e[:, pv_c0:],
                        start=(j == 0),
                        stop=(j == jmax),
                    )
                    # softmax denominator: accumulate e on DVE (PE was
                    # spending 512 cycles per block on a ones-matmul; one
                    # matmul per chunk over the accumulated e suffices)
                    a0 = P * r if r > 0 else 0
                    if j == 0:
                        nc.vector.tensor_copy(acc[:], e[:])
                    else:
                        nc.vector.tensor_add(
                            acc[:, a0:], acc[:, a0:], e[:, a0:]
                        )
                den_ps = ps_scr.tile([1, QCH], F32, tag="scr", name=f"den_ps{h}")
                nc.tensor.matmul(
                    den_ps[:], ones_sb[:], acc[:], start=True, stop=True
                )
                return out_ps, den_ps

            def attention_div(qc, h, out_ps, den_ps):
                q_sl = slice(qc * QCH, (qc + 1) * QCH)
                rd = small.tile([1, QCH], F32, tag="rd", name="rd")
                nc.vector.reciprocal(rd[:], den_ps[:])
                rdb = tmp.tile([P, QCH], F32, tag="rdb", name="rdb")
                nc.gpsimd.partition_broadcast(rdb[:], rd[:])
                nc.vector.tensor_mul(
                    attn_sb[h][:, q_sl], out_ps[:], rdb[:]
                )

            def do_wo(qc):
                """output projection for q chunk qc: bf16 partial staged in
                SBUF, written with a single DMA."""
                q_sl = slice(qc * QCH, (qc + 1) * QCH)
                stg = ostage.tile([P, KC * QCH], BF16, tag="stg", name="stg")
                last = qc == NQC - 1
                cuts = [8, 12, 14, 16] if last else [8, 16]
                done = 0
                for oc in range(KC):
                    if last:
                        o_ps = ps_att.tile([P, QCH], F32, tag="att", name="o_ps")
                    else:
                        o_ps = ps_wo.tile([P, QCH], F32, tag="wo", name="o_ps")
                    for c in range(2):
                        nc.tensor.matmul(
                            o_ps[:],
                            wo_sb[:, c, oc * P : (oc + 1) * P],
                            attn_sb[c][:, q_sl],
                            start=(c == 0),
                            stop=(c == 1),
                        )
                    if oc % 2 == 1:
                        # alternate copies onto ACT (Copy lives in every
                        # func-table set) to halve the serial PSUM drain
                        nc.scalar.copy(stg[:, oc * QCH : (oc + 1) * QCH], o_ps[:])
                    else:
                        nc.vector.tensor_copy(
                            stg[:, oc * QCH : (oc + 1) * QCH], o_ps[:]
                        )
                    if oc + 1 in cuts:
                        seg = slice(done * QCH, (oc + 1) * QCH)
                        nc.scalar.dma_start(out_d.ap()[qc, :, seg], stg[:, seg])
                        done = oc + 1

            def do_stats(p, psums):
                    # RMSNorm statistics (ACT Square/Sqrt run in the same
                    # pass, while ACT is otherwise idle — keeps them far from
                    # attention's Exp so func tables never thrash)
                    stats = []
                    for i in range(SCP):
                        ps = psums[i]
                        row = []
                        for hidx, c_sb, s_sb, dst in norm_specs:
                            off = hidx * P
                            sqv = tmp.tile([P, HEAD_DIM], F32, tag="sqv", name="sqv")
                            var = small.tile([P, 1], F32, tag="var", name="var")
                            nc.scalar.activation(
                                sqv[:], ps[:, off : off + P], AF.Square, accum_out=var[:]
                            )
                            lnv = small.tile([P, 1], F32, tag="lnv", name="lnv")
                            # rstd = exp(-0.5*ln(var/D + eps)): Ln and Exp
                            # share func-table set 6 with Square and Copy, so
                            # the whole kernel runs on ONE table set (a reload
                            # costs 1283ns and was thrashing at the
                            # projection->attention boundary)
                            nc.scalar.activation(
                                lnv[:], var[:], AF.Ln, scale=1.0 / HEAD_DIM, bias=eps_sb[:]
                            )
                            rstd = rstp.tile([P, 1], F32, tag="rstd", name="rstd")
                            nc.scalar.activation(rstd[:], lnv[:], AF.Exp, scale=-0.5)
                            row.append(rstd)
                        stats.append(row)
                    return stats

            def do_apply(p, psums, stats):
                    # RoPE + normalize + transpose to [d, s]; V copy (deferred
                    # one pass so the PE transposes don't head-of-line-block
                    # the next pass's projection matmuls)
                    for i in range(SCP):
                        sc = p * SCP + i
                        ps = psums[i]
                        for (hidx, c_sb, s_sb, dst), rstd in zip(norm_specs, stats[i]):
                            off = hidx * P
                            t1 = tmp.tile([P, HEAD_DIM], F32, tag="t1", name="t1")
                            nc.vector.scalar_tensor_tensor(
                                t1[:],
                                ps[:, off : off + P],
                                rstd[:],
                                c_sb[:, sc, :],
                                op0=OP.mult,
                                op1=OP.mult,
                            )
                            t2 = tmp.tile([P, HEAD_DIM], F32, tag="t2", name="t2")
                            nc.vector.scalar_tensor_tensor(
                                t2[:, 0:64],
                                ps[:, off + 64 : off + P],
                                rstd[:],
                                s_sb[:, sc, 0:64],
                                op0=OP.mult,
                                op1=OP.mult,
                            )
                            nc.vector.scalar_tensor_tensor(
                                t2[:, 64:P],
                                ps[:, off : off + 64],
                                rstd[:],
                                s_sb[:, sc, 64:P],
                                op0=OP.mult,
                                op1=OP.mult,
                            )
                            qsd = tmp.tile([P, HEAD_DIM], BF16, tag="qsd", name="qsd")
                            nc.vector.tensor_add(qsd[:], t1[:], t2[:])
                            tr = ps_scr.tile([P, P], BF16, tag="scr", name="tr")
                            nc.tensor.transpose(tr[:], qsd[:], ident[:])
                            nc.vector.tensor_copy(
                                dst[:, sc * P : (sc + 1) * P], tr[:]
                            )
                        # V: plain copy (cast) into [s, d] layout (DVE, so the
                        # Activation engine keeps one func-table set per phase)
                        nc.vector.tensor_copy(v_sb[:, sc, :], ps[:, 384:512])

            for rep in range(repeats):
                prev = None
                for p in range(NPASS):
                    if p == 0 and rep == 0:
                        nc.scalar.dma_start(w_sb[:, 0:KCG, :], wP.ap()[0])
                    hw_ = KC * PASS_W // 2
                    xt = stream.tile([P, KC * PASS_W], BF16, tag="xt", name="xt")
                    mt = stream.tile([P, KC * PASS_W], BF16, tag="mt", name="mt")
                    if p == 0 and rep == 0:
                        nc.sync.dma_start(xt[:, :hw_], xP.ap()[p, :, :hw_])
                        nc.sync.dma_start(xt[:, hw_:], xP.ap()[p, :, hw_:])
                        nc.sync.dma_start(mt[:, :hw_], muP.ap()[p, :, :hw_])
                        nc.sync.dma_start(mt[:, hw_:], muP.ap()[p, :, hw_:])
                    else:
                        nc.sync.dma_start(xt[:], xP.ap()[p])
                        nc.sync.dma_start(mt[:], muP.ap()[p])
                    if p == 0 and rep == 0:
                        for g in range(1, WG):
                            nc.scalar.dma_start(
                                w_sb[:, g * KCG : (g + 1) * KCG, :], wP.ap()[g]
                            )
                        for g in range(WG):
                            nc.scalar.dma_start(
                                wm_sb[:, g * KCG : (g + 1) * KCG, :], wmP.ap()[g]
                            )
                    psums = [
                        ps_att.tile([P, 512], F32, tag="att", name=f"proj{p}_{i}")
                        for i in range(SCP)
                    ]
                    # x @ W^T contributions
                    for kc in range(KC):
                        for i in range(SCP):
                            nc.tensor.matmul(
                                psums[i][:],
                                xt[:, kc * PASS_W + i * P : kc * PASS_W + (i + 1) * P],
                                w_sb[:, kc, :],
                                start=(kc == 0),
                                stop=False,
                            )
                    if p == 1 and rep == 0:
                        nc.scalar.dma_start(cq_sb[:], cosq.ap()[0])
                        nc.scalar.dma_start(sq_sb[:], sinq.ap()[0])
                        nc.scalar.dma_start(ck_sb[:], cosk.ap()[0])
                        nc.scalar.dma_start(sk_sb[:], sink.ap()[0])
                        for c in range(2):
                            nc.scalar.dma_start(wo_sb[:, c, :], woP.ap()[c])
                    # mu @ Wm^T contributions
                    for kc in range(KC):
                        for i in range(SCP):
                            nc.tensor.matmul(
                                psums[i][:],
                                mt[:, kc * PASS_W + i * P : kc * PASS_W + (i + 1) * P],
                                wm_sb[:, kc, :],
                                start=False,
                                stop=(kc == KC - 1),
                            )
                    stats = do_stats(p, psums)
                    if prev is not None:
                        do_apply(p - 1, *prev)
                    prev = (psums, stats)
                do_apply(NPASS - 1, *prev)
                # attention + output projection, after all projections
                # (keeps ACT on one table set per phase: sqrt/square first, exp after)
                for qc in range(NQC):
                    a0 = attention_head(qc, 0)
                    attention_div(qc, 0, *a0)
                    a1 = attention_head(qc, 1)
                    if qc > 0:
                        do_wo(qc - 1)
                    attention_div(qc, 1, *a1)
                do_wo(NQC - 1)


    nc.compile()
    return nc


def _get_program(repeats=1):
    if repeats not in _PROGRAM:
        _PROGRAM[repeats] = _build_program(repeats)
    return _PROGRAM[repeats]


def _host_prepare(inputs):
    """Shard + lay out inputs for the 8 cores."""
    hs = np.asarray(inputs["hidden_states"], dtype=np.float32).reshape(S, HID)
    mu = np.asarray(inputs["mu_prev"], dtype=np.float32).reshape(S, HID)
    wq = np.asarray(inputs["wq"], dtype=np.float32)
    wk = np.asarray(inputs["wk"], dtype=np.float32)
    wv = np.asarray(inputs["wv"], dtype=np.float32)
    wo = np.asarray(inputs["wo"], dtype=np.float32)
    wmq = np.asarray(inputs["wmq"], dtype=np.float32)
    wmk = np.asarray(inputs["wmk"], dtype=np.float32)
    wmv = np.asarray(inputs["wmv"], dtype=np.float32)
    qw = np.asarray(inputs["q_norm_w"], dtype=np.float32)
    kw = np.asarray(inputs["k_norm_w"], dtype=np.float32)

    def pack_stream(m):
        # [S, HID] -> [NPASS, P, KC*PASS_W]: per-pass blocks contiguous per
        # SBUF partition so each pass streams with one DMA.
        mT = np.ascontiguousarray(m.T).astype(NP_BF16)  # [HID, S]
        return np.ascontiguousarray(
            mT.reshape(KC, P, NPASS, PASS_W)
            .transpose(2, 1, 0, 3)
            .reshape(NPASS, P, KC * PASS_W)
        )

    xPh = pack_stream(hs)
    muPh = pack_stream(mu)

    # RoPE tables in [s, d] layout with rotate-half sign and norm weight baked in
    inv = 1.0 / (ROPE_THETA ** (np.arange(0, HEAD_DIM, 2, dtype=np.float32) / HEAD_DIM))
    ang = np.arange(S, dtype=np.float32)[:, None] * inv[None, :]  # [S, 64]
    emb = np.concatenate([ang, ang], axis=-1)  # [S, 128]
    cos_e = np.cos(emb)
    sin_e = np.sin(emb)
    sin_s = np.concatenate([-sin_e[:, :64], sin_e[:, 64:]], axis=-1)

    def tables(w):
        w_shift = np.concatenate([w[64:], w[:64]])
        cos_t = (cos_e * w[None, :]).astype(NP_BF16)
        sin_t = (sin_s * w_shift[None, :]).astype(NP_BF16)

        def pack(t):  # [S, D] -> [1, P, SC, D]
            return np.ascontiguousarray(
                t.reshape(SC, P, HEAD_DIM).transpose(1, 0, 2)[None]
            )

        return pack(cos_t), pack(sin_t)

    cq, sq = tables(qw)
    ck, sk = tables(kw)

    def pack_w(w_cat):
        # [HID, 512] -> [WG, P, KC//WG, 512] (per-group, per-partition
        # contiguous)
        return np.ascontiguousarray(
            w_cat.reshape(WG, KC // WG, P, 512).transpose(0, 2, 1, 3)
        ).astype(NP_BF16)

    in_maps = []
    for c in range(N_CORES):
        g = c // 2
        wq_s = wq[256 * c : 256 * (c + 1)]      # [256, HID]
        wmq_s = wmq[256 * c : 256 * (c + 1)]
        wk_s = wk[P * g : P * (g + 1)]          # [128, HID]
        wmk_s = wmk[P * g : P * (g + 1)]
        wv_s = wv[P * g : P * (g + 1)]
        wmv_s = wmv[P * g : P * (g + 1)]
        w_all = np.concatenate([wq_s.T, wk_s.T, wv_s.T], axis=1)     # [HID, 512]
        wm_all = np.concatenate([wmq_s.T, wmk_s.T, wmv_s.T], axis=1)
        woT_c = wo[:, 256 * c : 256 * (c + 1)].T                     # [256, HID]
        in_maps.append(
            {
                "xP": xPh,
                "muP": muPh,
                "wP": pack_w(w_all),
                "wmP": pack_w(wm_all),
                "woP": np.ascontiguousarray(woT_c).astype(NP_BF16).reshape(2, P, HID),
                "cosq": cq,
                "sinq": sq,
                "cosk": ck,
                "sink": sk,
            }
        )
    return in_maps


def run(inputs, trace=False):
    """Run the SPMD kernel; returns (full_output, exec_time_ns_or_None)."""
    nc = _get_program()
    in_maps = _host_prepare(inputs)
    res = run_bass_kernel_spmd(
        nc, in_maps, core_ids=list(range(N_CORES)), trace=trace
    )
    total = np.zeros((NQC, P, KC * QCH), dtype=np.float32)
    for c in range(N_CORES):
        total += np.asarray(res.results[c]["out"]).astype(np.float32)
    # [NQC, P, KC, QCH] -> [KC, P, NQC, QCH] -> [HID, S]
    full = (
        total.reshape(NQC, P, KC, QCH)
        .transpose(2, 1, 0, 3)
        .reshape(HID, S)
    )
    out = np.ascontiguousarray(full.T).reshape(B, S, HID).astype(np.float32)
    return out, res.exec_time_ns


def kernel(**inputs) -> np.ndarray:
    out, _ = run(inputs, trace=False)
    return out


# revision 14
# speedup vs baseline: 1.4646x; 1.0157x over previous
"""Trainium2 Bass kernel for nn_ComplexityAttention (GQA attention block).

Computation (B=1, S=2048, HID=2048, 16 Q heads / 4 KV heads, D=128):
  q/k/v = x @ W^T + mu @ Wm^T           (fused mu-guided projections)
  per-head RMSNorm on q, k; RoPE; causal GQA attention; out @ wo^T.

Sharding: tensor-parallel over heads across 8 NeuronCores. Core c owns
Q heads {2c, 2c+1} and KV head c//2 (KV work duplicated per core pair).
Each core produces a partial output (its heads' slice of wo applied),
host sums the 8 partials.

Device-side layout strategy:
  - Host pre-transposes x/mu to [HID, S] and packs them into per-pass
    blocks laid out contiguously per SBUF partition, so each projection
    pass streams with ONE large DMA (the shared HWDGE descriptor-gen
    device costs ~630ns per DMA instruction; many small DMAs starve PE).
  - Projections computed in [s, d] tiles (one PSUM bank holds q0|q1|k|v),
    RMSNorm+RoPE done with per-partition scalars + free-dim shifts
    (fused scalar_tensor_tensor ops), then Q/K PE-transposed to [d, s]
    for attention.
  - Scores computed transposed: S^T[kv, q] = K^T.T @ Q^T. Softmax without
    max-subtraction (scores bounded by +/-sqrt(128) after RMSNorm, exp is
    safe in fp32); denominator via ones-vector matmul; causal masking via
    a static multiplicative mask on the diagonal tiles. Fully-masked
    column ranges of partially-causal diagonal blocks are trimmed from
    the scores/exp/PV/den ops.
  - PV: out^T[d, q] = V[kv, d].T @ expS^T[kv, q] accumulated over kv chunks.
  - Output projection from out^T directly; partials staged in SBUF as
    bf16 and written with one DMA per 512-wide q chunk.

All matmul inputs are bf16 (fp32 PSUM accumulation); statistics in fp32.
"""

import sys

for _p in ("/opt/trn_rl_repo", "/root/.axon_site/_ro/trn_rl_repo"):
    if _p not in sys.path:
        sys.path.insert(0, _p)

import numpy as np
import ml_dtypes

import concourse.bass as bass
import concourse.bacc as bacc
import concourse.mybir as mybir
import concourse.hw_specs as hw_specs
import concourse.tile as tile
from concourse.bass_utils import run_bass_kernel_spmd
from concourse.masks import make_identity

# Problem constants (hardcoded per contract)
B, S, HID = 1, 2048, 2048
NUM_HEADS, NUM_KV_HEADS, HEAD_DIM = 16, 4, 128
ROPE_THETA = 10000.0
EPS = 1e-6
N_CORES = 8

P = 128
KC = HID // P            # 16 contraction chunks
SC = S // P              # 16 sequence chunks of 128
QCH = 512                # attention q-chunk (one PSUM bank)
NQC = S // QCH           # 4
NPASS = 8                # projection passes (2 s-chunks each)
SCP = SC // NPASS        # s-chunks per pass = 2
PASS_W = SCP * P         # 256 s columns per pass
WG = 8                   # weight-load groups (KC/WG kc chunks each)
QK_SCALE = 1.0 / float(np.sqrt(HEAD_DIM))

BF16 = mybir.dt.bfloat16
F32 = mybir.dt.float32
NP_BF16 = ml_dtypes.bfloat16

_PROGRAM = {}  # repeats -> compiled Bacc program

_ONE_SET = "natural_log_exp_and_others"  # act_info set 6: square+ln+exp+copy


def _pin_act_tables():
    """Make every activation func this kernel uses resolve ONLY to the
    natural_log_exp_and_others table set. The set-placement pass otherwise
    assigns Square/Exp to one set and Ln to another, inserting a 1283ns
    table reload between nearly every pair of activations."""
    if getattr(bacc, "_act_tables_pinned", False):
        return
    AF = mybir.ActivationFunctionType
    ours = {AF.Square, AF.Ln, AF.Exp, AF.Copy, AF.Identity}
    real_get = hw_specs.get_activation_tables

    def patched(arch):
        tabs = real_get(arch)
        return {
            name: (s if name == _ONE_SET else s - ours)
            for name, s in tabs.items()
        }

    bacc.get_activation_tables = patched
    bacc._act_tables_pinned = True


def _build_program(repeats=1):
    """Build the per-core Bass/Tile program (identical on all 8 cores)."""
    AF = mybir.ActivationFunctionType
    OP = mybir.AluOpType

    _pin_act_tables()
    nc = bacc.Bacc(trn_type="TRN2", debug=False)

    # ---- DRAM I/O (all pre-packed on host for single-DMA loads) ----
    xP = nc.dram_tensor("xP", [NPASS, P, KC * PASS_W], BF16, kind="ExternalInput")
    muP = nc.dram_tensor("muP", [NPASS, P, KC * PASS_W], BF16, kind="ExternalInput")
    # packed projection weights: [q0 | q1 | k | v] columns, [P, kc, 512] layout
    wP = nc.dram_tensor("wP", [WG, P, KC // WG, 512], BF16, kind="ExternalInput")
    wmP = nc.dram_tensor("wmP", [WG, P, KC // WG, 512], BF16, kind="ExternalInput")
    woP = nc.dram_tensor("woP", [2, P, HID], BF16, kind="ExternalInput")
    cosq = nc.dram_tensor("cosq", [1, P, SC, HEAD_DIM], BF16, kind="ExternalInput")
    sinq = nc.dram_tensor("sinq", [1, P, SC, HEAD_DIM], BF16, kind="ExternalInput")
    cosk = nc.dram_tensor("cosk", [1, P, SC, HEAD_DIM], BF16, kind="ExternalInput")
    sink = nc.dram_tensor("sink", [1, P, SC, HEAD_DIM], BF16, kind="ExternalInput")
    out_d = nc.dram_tensor("out", [NQC, P, KC * QCH], BF16, kind="ExternalOutput")

    KCG = KC // WG  # kc chunks per weight-load group

    with tile.TileContext(nc) as tc:
        with (
            tc.tile_pool(name="persist", bufs=1) as persist,
            tc.tile_pool(name="stream", bufs=3) as stream,
            tc.tile_pool(name="tmp", bufs=4) as tmp,
            tc.tile_pool(name="small", bufs=4) as small,
            tc.tile_pool(name="expp", bufs=6) as expp,
            tc.tile_pool(name="ostage", bufs=2) as ostage,
            tc.tile_pool(name="rstp", bufs=14) as rstp,
            tc.tile_pool(name="ps_att", bufs=4, space="PSUM") as ps_att,
            tc.tile_pool(name="ps_wo", bufs=2, space="PSUM") as ps_wo,
            tc.tile_pool(name="ps_scr", bufs=2, space="PSUM") as ps_scr,
        ):
            # ---- persistent SBUF tensors ----
            w_sb = persist.tile([P, KC, 512], BF16, name="w_sb")
            wm_sb = persist.tile([P, KC, 512], BF16, name="wm_sb")
            wo_sb = persist.tile([P, 2, HID], BF16, name="wo_sb")
            cq_sb = persist.tile([P, SC, HEAD_DIM], BF16, name="cq_sb")
            sq_sb = persist.tile([P, SC, HEAD_DIM], BF16, name="sq_sb")
            ck_sb = persist.tile([P, SC, HEAD_DIM], BF16, name="ck_sb")
            sk_sb = persist.tile([P, SC, HEAD_DIM], BF16, name="sk_sb")
            qt_sb = [
                persist.tile([P, S], BF16, name=f"qt{h}_sb") for h in range(2)
            ]
            kt_sb = persist.tile([P, S], BF16, name="kt_sb")
            v_sb = persist.tile([P, SC, HEAD_DIM], BF16, name="v_sb")
            attn_sb = [
                persist.tile([P, S], BF16, name=f"attn{c}_sb") for c in range(2)
            ]
            ident = persist.tile([P, P], BF16, name="ident")
            ones_sb = persist.tile([P, 1], BF16, name="ones_sb")
            eps_sb = persist.tile([P, 1], F32, name="eps_sb")
            diag_mask = persist.tile([P, P], BF16, name="diag_mask")

            make_identity(nc, ident[:])
            nc.gpsimd.memset(ones_sb[:], 1.0)
            nc.gpsimd.memset(eps_sb[:], EPS)
            # keep 1.0 where (q_local - kv_local) >= 0, else 0
            nc.gpsimd.memset(diag_mask[:], 1.0)
            nc.gpsimd.affine_select(
                out=diag_mask[:],
                in_=diag_mask[:],
                compare_op=mybir.AluOpType.is_ge,
                fill=0.0,
                base=0,
                pattern=[[1, P]],
                channel_multiplier=-1,
            )

            # head offsets inside the packed 512-wide projection output
            # (q0, q1, k occupy 0:128, 128:256, 256:384 and get norm+rope;
            #  v occupies 384:512)
            norm_specs = [
                (2, ck_sb, sk_sb, kt_sb),
                (0, cq_sb, sq_sb, qt_sb[0]),
                (1, cq_sb, sq_sb, qt_sb[1]),
            ]

            def attention_head(qc, h):
                """scores/exp/PV/den for one head of q chunk qc; returns
                (out_ps, den_ps). Heads are processed serially so the first
                head's PSUM frees mid-chunk and its softmax-div latency
                hides under the second head's matmuls."""
                jpq = QCH // P  # kv chunks per q chunk
                jmax = jpq * qc + (jpq - 1)
                out_ps = ps_att.tile([P, QCH], F32, tag="att", name=f"out_ps{h}")
                acc = tmp.tile([P, QCH], BF16, tag=f"acc{h}", name=f"acc{h}")
                for j in range(jmax + 1):
                    r = j - jpq * qc
                    # columns < P*r of this block are fully causal-masked.
                    # Trim them from scores/exp/PV/den, except on the last
                    # (stop) block where PV must cover the full PSUM
                    # width: there the dead columns of e are zeroed instead.
                    c0 = P * r if 0 < r < jpq - 1 else 0
                    s_ps = ps_att.tile([P, QCH], F32, tag="att", name="s_ps")
                    e = expp.tile([P, QCH], BF16, tag="e", name="e")
                    if r == jpq - 1:
                        nc.tensor.matmul(
                            s_ps[:, P * r :],
                            kt_sb[:, j * P : (j + 1) * P],
                            qt_sb[h][:, qc * QCH + P * r : (qc + 1) * QCH],
                            start=True,
                            stop=True,
                        )
                        nc.vector.memset(e[:, : P * r], 0.0)
                        nc.scalar.activation(
                            e[:, P * r :], s_ps[:, P * r :], AF.Exp,
                            scale=QK_SCALE,
                        )
                    else:
                        nc.tensor.matmul(
                            s_ps[:, c0:],
                            kt_sb[:, j * P : (j + 1) * P],
                            qt_sb[h][:, qc * QCH + c0 : (qc + 1) * QCH],
                            start=True,
                            stop=True,
                        )
                        nc.scalar.activation(
                            e[:, c0:], s_ps[:, c0:], AF.Exp, scale=QK_SCALE
                        )
                    if r >= 0:
                        # triangular mask on the 128-wide diagonal block
                        nc.vector.tensor_mul(
                            e[:, P * r : P * (r + 1)],
                            e[:, P * r : P * (r + 1)],
                            diag_mask[:],
                        )
                    pv_c0 = c0 if j != jmax else 0
                    nc.tensor.matmul(
                        out_ps[:, pv_c0:],
                        v_sb[:, j, :],
                        e[:, pv_c0:],
                        start=(j == 0),
                        stop=(j == jmax),
                    )
                    # softmax denominator: accumulate e on DVE (PE was
                    # spending 512 cycles per block on a ones-matmul; one
                    # matmul per chunk over the accumulated e suffices)
                    a0 = P * r if r > 0 else 0
                    if j == 0:
                        nc.vector.tensor_copy(acc[:], e[:])
                    else:
                        nc.vector.tensor_add(
                            acc[:, a0:], acc[:, a0:], e[:, a0:]
                        )
                den_ps = ps_scr.tile([1, QCH], F32, tag="scr", name=f"den_ps{h}")
                nc.tensor.matmul(
                    den_ps[:], ones_sb[:], acc[:], start=True, stop=True
                )
                return out_ps, den_ps

            def attention_div(qc, h, out_ps, den_ps):
                q_sl = slice(qc * QCH, (qc + 1) * QCH)
                rd = small.tile([1, QCH], F32, tag="rd", name="rd")
                nc.vector.reciprocal(rd[:], den_ps[:])
                rdb = tmp.tile([P, QCH], F32, tag="rdb", name="rdb")
                nc.gpsimd.partition_broadcast(rdb[:], rd[:])
                nc.vector.tensor_mul(
                    attn_sb[h][:, q_sl], out_ps[:], rdb[:]
                )

            def do_wo(qc):
                """output projection for q chunk qc: bf16 partial staged in
                SBUF, written with a single DMA."""
                q_sl = slice(qc * QCH, (qc + 1) * QCH)
                stg = ostage.tile([P, KC * QCH], BF16, tag="stg", name="stg")
                last = qc == NQC - 1
                cuts = [4, 8, 11, 14, 16] if last else [8, 16]
                done = 0
                for oc in range(KC):
                    if last:
                        o_ps = ps_att.tile([P, QCH], F32, tag="att", name="o_ps")
                    else:
                        o_ps = ps_wo.tile([P, QCH], F32, tag="wo", name="o_ps")
                    for c in range(2):
                        nc.tensor.matmul(
                            o_ps[:],
                            wo_sb[:, c, oc * P : (oc + 1) * P],
                            attn_sb[c][:, q_sl],
                            start=(c == 0),
                            stop=(c == 1),
                        )
                    if oc % 2 == 1:
                        # alternate copies onto ACT (Copy lives in every
                        # func-table set) to halve the serial PSUM drain
                        nc.scalar.copy(stg[:, oc * QCH : (oc + 1) * QCH], o_ps[:])
                    else:
                        nc.vector.tensor_copy(
                            stg[:, oc * QCH : (oc + 1) * QCH], o_ps[:]
                        )
                    if oc + 1 in cuts:
                        seg = slice(done * QCH, (oc + 1) * QCH)
                        nc.scalar.dma_start(out_d.ap()[qc, :, seg], stg[:, seg])
                        done = oc + 1

            def do_stats(p, psums):
                    # RMSNorm statistics (ACT Square/Sqrt run in the same
                    # pass, while ACT is otherwise idle — keeps them far from
                    # attention's Exp so func tables never thrash)
                    stats = []
                    for i in range(SCP):
                        ps = psums[i]
                        row = []
                        for hidx, c_sb, s_sb, dst in norm_specs:
                            off = hidx * P
                            sqv = tmp.tile([P, HEAD_DIM], F32, tag="sqv", name="sqv")
                            var = small.tile([P, 1], F32, tag="var", name="var")
                            nc.scalar.activation(
                                sqv[:], ps[:, off : off + P], AF.Square, accum_out=var[:]
                            )
                            lnv = small.tile([P, 1], F32, tag="lnv", name="lnv")
                            # rstd = exp(-0.5*ln(var/D + eps)): Ln and Exp
                            # share func-table set 6 with Square and Copy, so
                            # the whole kernel runs on ONE table set (a reload
                            # costs 1283ns and was thrashing at the
                            # projection->attention boundary)
                            nc.scalar.activation(
                                lnv[:], var[:], AF.Ln, scale=1.0 / HEAD_DIM, bias=eps_sb[:]
                            )
                            rstd = rstp.tile([P, 1], F32, tag="rstd", name="rstd")
                            nc.scalar.activation(rstd[:], lnv[:], AF.Exp, scale=-0.5)
                            row.append(rstd)
                        stats.append(row)
                    return stats

            def do_apply(p, psums, stats):
                    # RoPE + normalize + transpose to [d, s]; V copy (deferred
                    # one pass so the PE transposes don't head-of-line-block
                    # the next pass's projection matmuls)
                    for i in range(SCP):
                        sc = p * SCP + i
                        ps = psums[i]
                        for (hidx, c_sb, s_sb, dst), rstd in zip(norm_specs, stats[i]):
                            off = hidx * P
                            t1 = tmp.tile([P, HEAD_DIM], F32, tag="t1", name="t1")
                            nc.vector.scalar_tensor_tensor(
                                t1[:],
                                ps[:, off : off + P],
                                rstd[:],
                                c_sb[:, sc, :],
                                op0=OP.mult,
                                op1=OP.mult,
                            )
                            t2 = tmp.tile([P, HEAD_DIM], F32, tag="t2", name="t2")
                            nc.vector.scalar_tensor_tensor(
                                t2[:, 0:64],
                                ps[:, off + 64 : off + P],
                                rstd[:],
                                s_sb[:, sc, 0:64],
                                op0=OP.mult,
                                op1=OP.mult,
                            )
                            nc.vector.scalar_tensor_tensor(
                                t2[:, 64:P],
                                ps[:, off : off + 64],
                                rstd[:],
                                s_sb[:, sc, 64:P],
                                op0=OP.mult,
                                op1=OP.mult,
                            )
                            qsd = tmp.tile([P, HEAD_DIM], BF16, tag="qsd", name="qsd")
                            nc.vector.tensor_add(qsd[:], t1[:], t2[:])
                            tr = ps_scr.tile([P, P], BF16, tag="scr", name="tr")
                            nc.tensor.transpose(tr[:], qsd[:], ident[:])
                            nc.vector.tensor_copy(
                                dst[:, sc * P : (sc + 1) * P], tr[:]
                            )
                        # V: plain copy (cast) into [s, d] layout (DVE, so the
                        # Activation engine keeps one func-table set per phase)
                        nc.vector.tensor_copy(v_sb[:, sc, :], ps[:, 384:512])

            for rep in range(repeats):
                prev = None
                for p in range(NPASS):
                    if p == 0 and rep == 0:
                        nc.scalar.dma_start(w_sb[:, 0:KCG, :], wP.ap()[0])
                    hw_ = KC * PASS_W // 2
                    xt = stream.tile([P, KC * PASS_W], BF16, tag="xt", name="xt")
                    mt = stream.tile([P, KC * PASS_W], BF16, tag="mt", name="mt")
                    if p == 0 and rep == 0:
                        qw_ = hw_ // 2
                        for q4 in range(4):
                            nc.sync.dma_start(
                                xt[:, q4 * qw_ : (q4 + 1) * qw_],
                                xP.ap()[p, :, q4 * qw_ : (q4 + 1) * qw_],
                            )
                        nc.sync.dma_start(mt[:, :hw_], muP.ap()[p, :, :hw_])
                        nc.sync.dma_start(mt[:, hw_:], muP.ap()[p, :, hw_:])
                    else:
                        nc.sync.dma_start(xt[:], xP.ap()[p])
                        nc.sync.dma_start(mt[:], muP.ap()[p])
                    if p == 0 and rep == 0:
                        for g in range(1, WG):
                            nc.scalar.dma_start(
                                w_sb[:, g * KCG : (g + 1) * KCG, :], wP.ap()[g]
                            )
                        for g in range(WG):
                            nc.scalar.dma_start(
                                wm_sb[:, g * KCG : (g + 1) * KCG, :], wmP.ap()[g]
                            )
                    psums = [
                        ps_att.tile([P, 512], F32, tag="att", name=f"proj{p}_{i}")
                        for i in range(SCP)
                    ]
                    # x @ W^T contributions
                    for kc in range(KC):
                        for i in range(SCP):
                            nc.tensor.matmul(
                                psums[i][:],
                                xt[:, kc * PASS_W + i * P : kc * PASS_W + (i + 1) * P],
                                w_sb[:, kc, :],
                                start=(kc == 0),
                                stop=False,
                            )
                    if p == 1 and rep == 0:
                        nc.scalar.dma_start(cq_sb[:], cosq.ap()[0])
                        nc.scalar.dma_start(sq_sb[:], sinq.ap()[0])
                        nc.scalar.dma_start(ck_sb[:], cosk.ap()[0])
                        nc.scalar.dma_start(sk_sb[:], sink.ap()[0])
                    if p == 4 and rep == 0:
                        # wo is first read ~100us later (first do_wo): keep its
                        # load out of the congested startup DMA window
                        for c in range(2):
                            nc.scalar.dma_start(wo_sb[:, c, :], woP.ap()[c])
                    # mu @ Wm^T contributions
                    for kc in range(KC):
                        for i in range(SCP):
                            nc.tensor.matmul(
                                psums[i][:],
                                mt[:, kc * PASS_W + i * P : kc * PASS_W + (i + 1) * P],
                                wm_sb[:, kc, :],
                                start=False,
                                stop=(kc == KC - 1),
                            )
                    stats = do_stats(p, psums)
                    if prev is not None:
                        do_apply(p - 1, *prev)
                    prev = (psums, stats)
                do_apply(NPASS - 1, *prev)
                # attention + output projection, after all projections
                # (keeps ACT on one table set per phase: sqrt/square first, exp after)
                for qc in range(NQC):
                    a0 = attention_head(qc, 0)
                    attention_div(qc, 0, *a0)
                    a1 = attention_head(qc, 1)
                    if qc > 0:
                        do_wo(qc - 1)
                    attention_div(qc, 1, *a1)
                do_wo(NQC - 1)


    nc.compile()
    return nc


def _get_program(repeats=1):
    if repeats not in _PROGRAM:
        _PROGRAM[repeats] = _build_program(repeats)
    return _PROGRAM[repeats]


def _host_prepare(inputs):
    """Shard + lay out inputs for the 8 cores."""
    hs = np.asarray(inputs["hidden_states"], dtype=np.float32).reshape(S, HID)
    mu = np.asarray(inputs["mu_prev"], dtype=np.float32).reshape(S, HID)
    wq = np.asarray(inputs["wq"], dtype=np.float32)
    wk = np.asarray(inputs["wk"], dtype=np.float32)
    wv = np.asarray(inputs["wv"], dtype=np.float32)
    wo = np.asarray(inputs["wo"], dtype=np.float32)
    wmq = np.asarray(inputs["wmq"], dtype=np.float32)
    wmk = np.asarray(inputs["wmk"], dtype=np.float32)
    wmv = np.asarray(inputs["wmv"], dtype=np.float32)
    qw = np.asarray(inputs["q_norm_w"], dtype=np.float32)
    kw = np.asarray(inputs["k_norm_w"], dtype=np.float32)

    def pack_stream(m):
        # [S, HID] -> [NPASS, P, KC*PASS_W]: per-pass blocks contiguous per
        # SBUF partition so each pass streams with one DMA.
        mT = np.ascontiguousarray(m.T).astype(NP_BF16)  # [HID, S]
        return np.ascontiguousarray(
            mT.reshape(KC, P, NPASS, PASS_W)
            .transpose(2, 1, 0, 3)
            .reshape(NPASS, P, KC * PASS_W)
        )

    xPh = pack_stream(hs)
    muPh = pack_stream(mu)

    # RoPE tables in [s, d] layout with rotate-half sign and norm weight baked in
    inv = 1.0 / (ROPE_THETA ** (np.arange(0, HEAD_DIM, 2, dtype=np.float32) / HEAD_DIM))
    ang = np.arange(S, dtype=np.float32)[:, None] * inv[None, :]  # [S, 64]
    emb = np.concatenate([ang, ang], axis=-1)  # [S, 128]
    cos_e = np.cos(emb)
    sin_e = np.sin(emb)
    sin_s = np.concatenate([-sin_e[:, :64], sin_e[:, 64:]], axis=-1)

    def tables(w):
        w_shift = np.concatenate([w[64:], w[:64]])
        cos_t = (cos_e * w[None, :]).astype(NP_BF16)
        sin_t = (sin_s * w_shift[None, :]).astype(NP_BF16)

        def pack(t):  # [S, D] -> [1, P, SC, D]
            return np.ascontiguousarray(
                t.reshape(SC, P, HEAD_DIM).transpose(1, 0, 2)[None]
            )

        return pack(cos_t), pack(sin_t)

    cq, sq = tables(qw)
    ck, sk = tables(kw)

    def pack_w(w_cat):
        # [HID, 512] -> [WG, P, KC//WG, 512] (per-group, per-partition
        # contiguous)
        return np.ascontiguousarray(
            w_cat.reshape(WG, KC // WG, P, 512).transpose(0, 2, 1, 3)
        ).astype(NP_BF16)

    in_maps = []
    for c in range(N_CORES):
        g = c // 2
        wq_s = wq[256 * c : 256 * (c + 1)]      # [256, HID]
        wmq_s = wmq[256 * c : 256 * (c + 1)]
        wk_s = wk[P * g : P * (g + 1)]          # [128, HID]
        wmk_s = wmk[P * g : P * (g + 1)]
        wv_s = wv[P * g : P * (g + 1)]
        wmv_s = wmv[P * g : P * (g + 1)]
        w_all = np.concatenate([wq_s.T, wk_s.T, wv_s.T], axis=1)     # [HID, 512]
        wm_all = np.concatenate([wmq_s.T, wmk_s.T, wmv_s.T], axis=1)
        woT_c = wo[:, 256 * c : 256 * (c + 1)].T                     # [256, HID]
        in_maps.append(
            {
                "xP": xPh,
                "muP": muPh,
                "wP": pack_w(w_all),
                "wmP": pack_w(wm_all),
                "woP": np.ascontiguousarray(woT_c).astype(NP_BF16).reshape(2, P, HID),
                "cosq": cq,
                "sinq": sq,
                "cosk": ck,
                "sink": sk,
            }
        )
    return in_maps


def run(inputs, trace=False):
    """Run the SPMD kernel; returns (full_output, exec_time_ns_or_None)."""
    nc = _get_program()
    in_maps = _host_prepare(inputs)
    res = run_bass_kernel_spmd(
        nc, in_maps, core_ids=list(range(N_CORES)), trace=trace
    )
    total = np.zeros((NQC, P, KC * QCH), dtype=np.float32)
    for c in range(N_CORES):
        total += np.asarray(res.results[c]["out"]).astype(np.float32)
    # [NQC, P, KC, QCH] -> [KC, P, NQC, QCH] -> [HID, S]
    full = (
        total.reshape(NQC, P, KC, QCH)
        .transpose(2, 1, 0, 3)
        .reshape(HID, S)
    )
    out = np.ascontiguousarray(full.T).reshape(B, S, HID).astype(np.float32)
    return out, res.exec_time_ns


def kernel(**inputs) -> np.ndarray:
    out, _ = run(inputs, trace=False)
    return out


# revision 18
# speedup vs baseline: 1.5192x; 1.0373x over previous
"""Trainium2 Bass kernel for nn_ComplexityAttention (GQA attention block).

Computation (B=1, S=2048, HID=2048, 16 Q heads / 4 KV heads, D=128):
  q/k/v = x @ W^T + mu @ Wm^T           (fused mu-guided projections)
  per-head RMSNorm on q, k; RoPE; causal GQA attention; out @ wo^T.

Sharding: tensor-parallel over heads across 8 NeuronCores. Core c owns
Q heads {2c, 2c+1} and KV head c//2 (KV work duplicated per core pair).
Each core produces a partial output (its heads' slice of wo applied),
host sums the 8 partials.

Device-side layout strategy:
  - Host pre-transposes x/mu to [HID, S] and packs them into per-pass
    blocks laid out contiguously per SBUF partition, so each projection
    pass streams with ONE large DMA (the shared HWDGE descriptor-gen
    device costs ~630ns per DMA instruction; many small DMAs starve PE).
  - Projections computed in [s, d] tiles (one PSUM bank holds q0|q1|k|v),
    RMSNorm+RoPE done with per-partition scalars + free-dim shifts
    (fused scalar_tensor_tensor ops), then Q/K PE-transposed to [d, s]
    for attention.
  - Scores computed transposed: S^T[kv, q] = K^T.T @ Q^T. Softmax without
    max-subtraction (scores bounded by +/-sqrt(128) after RMSNorm, exp is
    safe in fp32); denominator via ones-vector matmul; causal masking via
    a static multiplicative mask on the diagonal tiles. Fully-masked
    column ranges of partially-causal diagonal blocks are trimmed from
    the scores/exp/PV/den ops.
  - PV: out^T[d, q] = V[kv, d].T @ expS^T[kv, q] accumulated over kv chunks.
  - Output projection from out^T directly; partials staged in SBUF as
    bf16 and written with one DMA per 512-wide q chunk.

All matmul inputs are bf16 (fp32 PSUM accumulation); statistics in fp32.
"""

import sys

for _p in ("/opt/trn_rl_repo", "/root/.axon_site/_ro/trn_rl_repo"):
    if _p not in sys.path:
        sys.path.insert(0, _p)

import numpy as np
import ml_dtypes

import concourse.bass as bass
import concourse.bacc as bacc
import concourse.mybir as mybir
import concourse.hw_specs as hw_specs
import concourse.tile as tile
from concourse.bass_utils import run_bass_kernel_spmd
from concourse.masks import make_identity

# Problem constants (hardcoded per contract)
B, S, HID = 1, 2048, 2048
NUM_HEADS, NUM_KV_HEADS, HEAD_DIM = 16, 4, 128
ROPE_THETA = 10000.0
EPS = 1e-6
N_CORES = 8

P = 128
KC = HID // P            # 16 contraction chunks
SC = S // P              # 16 sequence chunks of 128
QCH = 512                # attention q-chunk (one PSUM bank)
NQC = S // QCH           # 4
NPASS = 8                # projection passes (2 s-chunks each)
SCP = SC // NPASS        # s-chunks per pass = 2
PASS_W = SCP * P         # 256 s columns per pass
WG = 8                   # weight-load groups (KC/WG kc chunks each)
QK_SCALE = 1.0 / float(np.sqrt(HEAD_DIM))

BF16 = mybir.dt.bfloat16
F32 = mybir.dt.float32
NP_BF16 = ml_dtypes.bfloat16

_PROGRAM = {}  # repeats -> compiled Bacc program

_ONE_SET = "natural_log_exp_and_others"  # act_info set 6: square+ln+exp+copy


def _pin_act_tables():
    """Make every activation func this kernel uses resolve ONLY to the
    natural_log_exp_and_others table set. The set-placement pass otherwise
    assigns Square/Exp to one set and Ln to another, inserting a 1283ns
    table reload between nearly every pair of activations."""
    if getattr(bacc, "_act_tables_pinned", False):
        return
    AF = mybir.ActivationFunctionType
    ours = {AF.Square, AF.Ln, AF.Exp, AF.Copy, AF.Identity}
    real_get = hw_specs.get_activation_tables

    def patched(arch):
        tabs = real_get(arch)
        return {
            name: (s if name == _ONE_SET else s - ours)
            for name, s in tabs.items()
        }

    bacc.get_activation_tables = patched
    bacc._act_tables_pinned = True


def _build_program(repeats=1):
    """Build the per-core Bass/Tile program (identical on all 8 cores)."""
    AF = mybir.ActivationFunctionType
    OP = mybir.AluOpType

    _pin_act_tables()
    nc = bacc.Bacc(trn_type="TRN2", debug=False)

    # ---- DRAM I/O (all pre-packed on host for single-DMA loads) ----
    xP = nc.dram_tensor("xP", [NPASS, P, KC * PASS_W], BF16, kind="ExternalInput")
    muP = nc.dram_tensor("muP", [NPASS, P, KC * PASS_W], BF16, kind="ExternalInput")
    # packed projection weights: [q0 | q1 | k | v] columns, [P, kc, 512] layout
    wP = nc.dram_tensor("wP", [WG, P, KC // WG, 512], BF16, kind="ExternalInput")
    wmP = nc.dram_tensor("wmP", [WG, P, KC // WG, 512], BF16, kind="ExternalInput")
    woP = nc.dram_tensor("woP", [2, P, HID], BF16, kind="ExternalInput")
    cosq = nc.dram_tensor("cosq", [1, P, SC, HEAD_DIM], BF16, kind="ExternalInput")
    sinq = nc.dram_tensor("sinq", [1, P, SC, HEAD_DIM], BF16, kind="ExternalInput")
    cosk = nc.dram_tensor("cosk", [1, P, SC, HEAD_DIM], BF16, kind="ExternalInput")
    sink = nc.dram_tensor("sink", [1, P, SC, HEAD_DIM], BF16, kind="ExternalInput")
    out_d = nc.dram_tensor("out", [NQC, P, KC * QCH], BF16, kind="ExternalOutput")

    KCG = KC // WG  # kc chunks per weight-load group

    with tile.TileContext(nc) as tc:
        with (
            tc.tile_pool(name="persist", bufs=1) as persist,
            tc.tile_pool(name="stream", bufs=3) as stream,
            tc.tile_pool(name="tmp", bufs=4) as tmp,
            tc.tile_pool(name="small", bufs=4) as small,
            tc.tile_pool(name="expp", bufs=6) as expp,
            tc.tile_pool(name="ostage", bufs=2) as ostage,
            tc.tile_pool(name="rstp", bufs=14) as rstp,
            tc.tile_pool(name="ps_att", bufs=4, space="PSUM") as ps_att,
            tc.tile_pool(name="ps_wo", bufs=2, space="PSUM") as ps_wo,
            tc.tile_pool(name="ps_scr", bufs=2, space="PSUM") as ps_scr,
        ):
            # ---- persistent SBUF tensors ----
            w_sb = persist.tile([P, KC, 512], BF16, name="w_sb")
            wm_sb = persist.tile([P, KC, 512], BF16, name="wm_sb")
            wo_sb = persist.tile([P, 2, HID], BF16, name="wo_sb")
            cq_sb = persist.tile([P, SC, HEAD_DIM], BF16, name="cq_sb")
            sq_sb = persist.tile([P, SC, HEAD_DIM], BF16, name="sq_sb")
            ck_sb = persist.tile([P, SC, HEAD_DIM], BF16, name="ck_sb")
            sk_sb = persist.tile([P, SC, HEAD_DIM], BF16, name="sk_sb")
            qt_sb = [
                persist.tile([P, S], BF16, name=f"qt{h}_sb") for h in range(2)
            ]
            kt_sb = persist.tile([P, S], BF16, name="kt_sb")
            v_sb = persist.tile([P, SC, HEAD_DIM], BF16, name="v_sb")
            attn_sb = [
                persist.tile([P, S], BF16, name=f"attn{c}_sb") for c in range(2)
            ]
            ident = persist.tile([P, P], BF16, name="ident")
            ones_sb = persist.tile([P, 1], BF16, name="ones_sb")
            eps_sb = persist.tile([P, 1], F32, name="eps_sb")
            diag_mask = persist.tile([P, P], BF16, name="diag_mask")

            make_identity(nc, ident[:])
            nc.gpsimd.memset(ones_sb[:], 1.0)
            nc.gpsimd.memset(eps_sb[:], EPS)
            # keep 1.0 where (q_local - kv_local) >= 0, else 0
            nc.gpsimd.memset(diag_mask[:], 1.0)
            nc.gpsimd.affine_select(
                out=diag_mask[:],
                in_=diag_mask[:],
                compare_op=mybir.AluOpType.is_ge,
                fill=0.0,
                base=0,
                pattern=[[1, P]],
                channel_multiplier=-1,
            )

            # head offsets inside the packed 512-wide projection output
            # (q0, q1, k occupy 0:128, 128:256, 256:384 and get norm+rope;
            #  v occupies 384:512)
            norm_specs = [
                (2, ck_sb, sk_sb, kt_sb),
                (0, cq_sb, sq_sb, qt_sb[0]),
                (1, cq_sb, sq_sb, qt_sb[1]),
            ]

            def attention_head(qc, h):
                """scores/exp/PV/den for one head of q chunk qc; returns
                (out_ps, den_ps). Heads are processed serially so the first
                head's PSUM frees mid-chunk and its softmax-div latency
                hides under the second head's matmuls."""
                jpq = QCH // P  # kv chunks per q chunk
                jmax = jpq * qc + (jpq - 1)
                out_ps = ps_att.tile([P, QCH], F32, tag="att", name=f"out_ps{h}")
                acc = tmp.tile([P, QCH], BF16, tag=f"acc{h}", name=f"acc{h}")
                for j in range(jmax + 1):
                    r = j - jpq * qc
                    # columns < P*r of this block are fully causal-masked.
                    # Trim them from scores/exp/PV/den, except on the last
                    # (stop) block where PV must cover the full PSUM
                    # width: there the dead columns of e are zeroed instead.
                    c0 = P * r if 0 < r < jpq - 1 else 0
                    s_ps = ps_att.tile([P, QCH], F32, tag="att", name="s_ps")
                    e = expp.tile([P, QCH], BF16, tag="e", name="e")
                    if r == jpq - 1:
                        nc.tensor.matmul(
                            s_ps[:, P * r :],
                            kt_sb[:, j * P : (j + 1) * P],
                            qt_sb[h][:, qc * QCH + P * r : (qc + 1) * QCH],
                            start=True,
                            stop=True,
                        )
                        nc.vector.memset(e[:, : P * r], 0.0)
                        nc.scalar.activation(
                            e[:, P * r :], s_ps[:, P * r :], AF.Exp,
                            scale=QK_SCALE,
                        )
                    else:
                        nc.tensor.matmul(
                            s_ps[:, c0:],
                            kt_sb[:, j * P : (j + 1) * P],
                            qt_sb[h][:, qc * QCH + c0 : (qc + 1) * QCH],
                            start=True,
                            stop=True,
                        )
                        nc.scalar.activation(
                            e[:, c0:], s_ps[:, c0:], AF.Exp, scale=QK_SCALE
                        )
                    if r >= 0:
                        # triangular mask on the 128-wide diagonal block
                        nc.vector.tensor_mul(
                            e[:, P * r : P * (r + 1)],
                            e[:, P * r : P * (r + 1)],
                            diag_mask[:],
                        )
                    pv_c0 = c0 if j != jmax else 0
                    nc.tensor.matmul(
                        out_ps[:, pv_c0:],
                        v_sb[:, j, :],
                        e[:, pv_c0:],
                        start=(j == 0),
                        stop=(j == jmax),
                    )
                    # softmax denominator: accumulate e on DVE (PE was
                    # spending 512 cycles per block on a ones-matmul; one
                    # matmul per chunk over the accumulated e suffices)
                    a0 = P * r if r > 0 else 0
                    if j == 0:
                        nc.vector.tensor_copy(acc[:], e[:])
                    else:
                        nc.vector.tensor_add(
                            acc[:, a0:], acc[:, a0:], e[:, a0:]
                        )
                return out_ps, acc

            def attention_finish(qc, h, out_ps, acc):
                # den matmul waits on the DVE exp-accumulate chain; emitted
                # after BOTH heads' block streams so it never stalls the PE
                den_ps = ps_scr.tile([1, QCH], F32, tag="scr", name=f"den_ps{h}")
                nc.tensor.matmul(
                    den_ps[:], ones_sb[:], acc[:], start=True, stop=True
                )
                attention_div(qc, h, out_ps, den_ps)

            def attention_div(qc, h, out_ps, den_ps):
                q_sl = slice(qc * QCH, (qc + 1) * QCH)
                rd = small.tile([1, QCH], F32, tag="rd", name="rd")
                nc.vector.reciprocal(rd[:], den_ps[:])
                rdb = tmp.tile([P, QCH], F32, tag="rdb", name="rdb")
                nc.gpsimd.partition_broadcast(rdb[:], rd[:])
                nc.vector.tensor_mul(
                    attn_sb[h][:, q_sl], out_ps[:], rdb[:]
                )

            def do_wo(qc):
                """output projection for q chunk qc: bf16 partial staged in
                SBUF, written with a single DMA."""
                q_sl = slice(qc * QCH, (qc + 1) * QCH)
                stg = ostage.tile([P, KC * QCH], BF16, tag="stg", name="stg")
                last = qc == NQC - 1
                cuts = [4, 8, 11, 14, 16] if last else [8, 16]
                done = 0
                for oc in range(KC):
                    if last:
                        o_ps = ps_att.tile([P, QCH], F32, tag="att", name="o_ps")
                    else:
                        o_ps = ps_wo.tile([P, QCH], F32, tag="wo", name="o_ps")
                    for c in range(2):
                        nc.tensor.matmul(
                            o_ps[:],
                            wo_sb[:, c, oc * P : (oc + 1) * P],
                            attn_sb[c][:, q_sl],
                            start=(c == 0),
                            stop=(c == 1),
                        )
                    if oc % 2 == 1:
                        # alternate copies onto ACT (Copy lives in every
                        # func-table set) to halve the serial PSUM drain
                        nc.scalar.copy(stg[:, oc * QCH : (oc + 1) * QCH], o_ps[:])
                    else:
                        nc.vector.tensor_copy(
                            stg[:, oc * QCH : (oc + 1) * QCH], o_ps[:]
                        )
                    if oc + 1 in cuts:
                        seg = slice(done * QCH, (oc + 1) * QCH)
                        nc.scalar.dma_start(out_d.ap()[qc, :, seg], stg[:, seg])
                        done = oc + 1

            def do_stats(p, psums):
                    # RMSNorm statistics (ACT Square/Sqrt run in the same
                    # pass, while ACT is otherwise idle — keeps them far from
                    # attention's Exp so func tables never thrash)
                    stats = []
                    for i in range(SCP):
                        ps = psums[i]
                        row = []
                        for hidx, c_sb, s_sb, dst in norm_specs:
                            off = hidx * P
                            sqv = tmp.tile([P, HEAD_DIM], F32, tag="sqv", name="sqv")
                            var = small.tile([P, 1], F32, tag="var", name="var")
                            nc.scalar.activation(
                                sqv[:], ps[:, off : off + P], AF.Square, accum_out=var[:]
                            )
                            lnv = small.tile([P, 1], F32, tag="lnv", name="lnv")
                            # rstd = exp(-0.5*ln(var/D + eps)): Ln and Exp
                            # share func-table set 6 with Square and Copy, so
                            # the whole kernel runs on ONE table set (a reload
                            # costs 1283ns and was thrashing at the
                            # projection->attention boundary)
                            nc.scalar.activation(
                                lnv[:], var[:], AF.Ln, scale=1.0 / HEAD_DIM, bias=eps_sb[:]
                            )
                            rstd = rstp.tile([P, 1], F32, tag="rstd", name="rstd")
                            nc.scalar.activation(rstd[:], lnv[:], AF.Exp, scale=-0.5)
                            row.append(rstd)
                        stats.append(row)
                    return stats

            def do_apply(p, psums, stats):
                    # RoPE + normalize + transpose to [d, s]; V copy (deferred
                    # one pass so the PE transposes don't head-of-line-block
                    # the next pass's projection matmuls)
                    for i in range(SCP):
                        sc = p * SCP + i
                        ps = psums[i]
                        for (hidx, c_sb, s_sb, dst), rstd in zip(norm_specs, stats[i]):
                            off = hidx * P
                            t1 = tmp.tile([P, HEAD_DIM], F32, tag="t1", name="t1")
                            nc.vector.scalar_tensor_tensor(
                                t1[:],
                                ps[:, off : off + P],
                                rstd[:],
                                c_sb[:, sc, :],
                                op0=OP.mult,
                                op1=OP.mult,
                            )
                            t2 = tmp.tile([P, HEAD_DIM], F32, tag="t2", name="t2")
                            nc.vector.scalar_tensor_tensor(
                                t2[:, 0:64],
                                ps[:, off + 64 : off + P],
                                rstd[:],
                                s_sb[:, sc, 0:64],
                                op0=OP.mult,
                                op1=OP.mult,
                            )
                            nc.vector.scalar_tensor_tensor(
                                t2[:, 64:P],
                                ps[:, off : off + 64],
                                rstd[:],
                                s_sb[:, sc, 64:P],
                                op0=OP.mult,
                                op1=OP.mult,
                            )
                            qsd = tmp.tile([P, HEAD_DIM], BF16, tag="qsd", name="qsd")
                            nc.vector.tensor_add(qsd[:], t1[:], t2[:])
                            tr = ps_scr.tile([P, P], BF16, tag="scr", name="tr")
                            nc.tensor.transpose(tr[:], qsd[:], ident[:])
                            nc.vector.tensor_copy(
                                dst[:, sc * P : (sc + 1) * P], tr[:]
                            )
                        # V: plain copy (cast) into [s, d] layout (DVE, so the
                        # Activation engine keeps one func-table set per phase)
                        nc.vector.tensor_copy(v_sb[:, sc, :], ps[:, 384:512])

            for rep in range(repeats):
                prev = None
                for p in range(NPASS):
                    if p == 0 and rep == 0:
                        nc.scalar.dma_start(w_sb[:, 0:KCG, :], wP.ap()[0])
                    hw_ = KC * PASS_W // 2
                    xt = stream.tile([P, KC * PASS_W], BF16, tag="xt", name="xt")
                    mt = stream.tile([P, KC * PASS_W], BF16, tag="mt", name="mt")
                    if p == 0 and rep == 0:
                        qw_ = hw_ // 2
                        for q4 in range(4):
                            nc.sync.dma_start(
                                xt[:, q4 * qw_ : (q4 + 1) * qw_],
                                xP.ap()[p, :, q4 * qw_ : (q4 + 1) * qw_],
                            )
                        nc.sync.dma_start(mt[:, :hw_], muP.ap()[p, :, :hw_])
                        nc.sync.dma_start(mt[:, hw_:], muP.ap()[p, :, hw_:])
                    else:
                        nc.sync.dma_start(xt[:], xP.ap()[p])
                        nc.sync.dma_start(mt[:], muP.ap()[p])
                    if p == 0 and rep == 0:
                        for g in range(1, WG):
                            nc.scalar.dma_start(
                                w_sb[:, g * KCG : (g + 1) * KCG, :], wP.ap()[g]
                            )
                        for g in range(WG):
                            nc.scalar.dma_start(
                                wm_sb[:, g * KCG : (g + 1) * KCG, :], wmP.ap()[g]
                            )
                    psums = [
                        ps_att.tile([P, 512], F32, tag="att", name=f"proj{p}_{i}")
                        for i in range(SCP)
                    ]
                    # x @ W^T contributions
                    for kc in range(KC):
                        for i in range(SCP):
                            nc.tensor.matmul(
                                psums[i][:],
                                xt[:, kc * PASS_W + i * P : kc * PASS_W + (i + 1) * P],
                                w_sb[:, kc, :],
                                start=(kc == 0),
                                stop=False,
                            )
                    if p == 1 and rep == 0:
                        h8 = SC // 2
                        nc.scalar.dma_start(cq_sb[:, :h8, :], cosq.ap()[0, :, :h8])
                        nc.scalar.dma_start(sq_sb[:, :h8, :], sinq.ap()[0, :, :h8])
                        nc.scalar.dma_start(ck_sb[:, :h8, :], cosk.ap()[0, :, :h8])
                        nc.scalar.dma_start(sk_sb[:, :h8, :], sink.ap()[0, :, :h8])
                    if p == 3 and rep == 0:
                        h8 = SC // 2
                        nc.scalar.dma_start(cq_sb[:, h8:, :], cosq.ap()[0, :, h8:])
                        nc.scalar.dma_start(sq_sb[:, h8:, :], sinq.ap()[0, :, h8:])
                        nc.scalar.dma_start(ck_sb[:, h8:, :], cosk.ap()[0, :, h8:])
                        nc.scalar.dma_start(sk_sb[:, h8:, :], sink.ap()[0, :, h8:])
                    if p == 4 and rep == 0:
                        # wo is first read ~100us later (first do_wo): keep its
                        # load out of the congested startup DMA window
                        for c in range(2):
                            nc.scalar.dma_start(wo_sb[:, c, :], woP.ap()[c])
                    # mu @ Wm^T contributions
                    for kc in range(KC):
                        for i in range(SCP):
                            nc.tensor.matmul(
                                psums[i][:],
                                mt[:, kc * PASS_W + i * P : kc * PASS_W + (i + 1) * P],
                                wm_sb[:, kc, :],
                                start=False,
                                stop=(kc == KC - 1),
                            )
                    stats = do_stats(p, psums)
                    if prev is not None:
                        do_apply(p - 1, *prev)
                    prev = (psums, stats)
                do_apply(NPASS - 1, *prev)
                # attention + output projection, after all projections
                # (keeps ACT on one table set per phase: sqrt/square first, exp after)
                for qc in range(NQC):
                    a0 = attention_head(qc, 0)
                    a1 = attention_head(qc, 1)
                    attention_finish(qc, 0, *a0)
                    if qc > 0:
                        do_wo(qc - 1)
                    attention_finish(qc, 1, *a1)
                do_wo(NQC - 1)


    nc.compile()
    return nc


def _get_program(repeats=1):
    if repeats not in _PROGRAM:
        _PROGRAM[repeats] = _build_program(repeats)
    return _PROGRAM[repeats]


def _host_prepare(inputs):
    """Shard + lay out inputs for the 8 cores."""
    hs = np.asarray(inputs["hidden_states"], dtype=np.float32).reshape(S, HID)
    mu = np.asarray(inputs["mu_prev"], dtype=np.float32).reshape(S, HID)
    wq = np.asarray(inputs["wq"], dtype=np.float32)
    wk = np.asarray(inputs["wk"], dtype=np.float32)
    wv = np.asarray(inputs["wv"], dtype=np.float32)
    wo = np.asarray(inputs["wo"], dtype=np.float32)
    wmq = np.asarray(inputs["wmq"], dtype=np.float32)
    wmk = np.asarray(inputs["wmk"], dtype=np.float32)
    wmv = np.asarray(inputs["wmv"], dtype=np.float32)
    qw = np.asarray(inputs["q_norm_w"], dtype=np.float32)
    kw = np.asarray(inputs["k_norm_w"], dtype=np.float32)

    def pack_stream(m):
        # [S, HID] -> [NPASS, P, KC*PASS_W]: per-pass blocks contiguous per
        # SBUF partition so each pass streams with one DMA.
        mT = np.ascontiguousarray(m.T).astype(NP_BF16)  # [HID, S]
        return np.ascontiguousarray(
            mT.reshape(KC, P, NPASS, PASS_W)
            .transpose(2, 1, 0, 3)
            .reshape(NPASS, P, KC * PASS_W)
        )

    xPh = pack_stream(hs)
    muPh = pack_stream(mu)

    # RoPE tables in [s, d] layout with rotate-half sign and norm weight baked in
    inv = 1.0 / (ROPE_THETA ** (np.arange(0, HEAD_DIM, 2, dtype=np.float32) / HEAD_DIM))
    ang = np.arange(S, dtype=np.float32)[:, None] * inv[None, :]  # [S, 64]
    emb = np.concatenate([ang, ang], axis=-1)  # [S, 128]
    cos_e = np.cos(emb)
    sin_e = np.sin(emb)
    sin_s = np.concatenate([-sin_e[:, :64], sin_e[:, 64:]], axis=-1)

    def tables(w):
        w_shift = np.concatenate([w[64:], w[:64]])
        cos_t = (cos_e * w[None, :]).astype(NP_BF16)
        sin_t = (sin_s * w_shift[None, :]).astype(NP_BF16)

        def pack(t):  # [S, D] -> [1, P, SC, D]
            return np.ascontiguousarray(
                t.reshape(SC, P, HEAD_DIM).transpose(1, 0, 2)[None]
            )

        return pack(cos_t), pack(sin_t)

    cq, sq = tables(qw)
    ck, sk = tables(kw)

    def pack_w(w_cat):
        # [HID, 512] -> [WG, P, KC//WG, 512] (per-group, per-partition
        # contiguous)
        return np.ascontiguousarray(
            w_cat.reshape(WG, KC // WG, P, 512).transpose(0, 2, 1, 3)
        ).astype(NP_BF16)

    in_maps = []
    for c in range(N_CORES):
        g = c // 2
        wq_s = wq[256 * c : 256 * (c + 1)]      # [256, HID]
        wmq_s = wmq[256 * c : 256 * (c + 1)]
        wk_s = wk[P * g : P * (g + 1)]          # [128, HID]
        wmk_s = wmk[P * g : P * (g + 1)]
        wv_s = wv[P * g : P * (g + 1)]
        wmv_s = wmv[P * g : P * (g + 1)]
        w_all = np.concatenate([wq_s.T, wk_s.T, wv_s.T], axis=1)     # [HID, 512]
        wm_all = np.concatenate([wmq_s.T, wmk_s.T, wmv_s.T], axis=1)
        woT_c = wo[:, 256 * c : 256 * (c + 1)].T                     # [256, HID]
        in_maps.append(
            {
                "xP": xPh,
                "muP": muPh,
                "wP": pack_w(w_all),
                "wmP": pack_w(wm_all),
                "woP": np.ascontiguousarray(woT_c).astype(NP_BF16).reshape(2, P, HID),
                "cosq": cq,
                "sinq": sq,
                "cosk": ck,
                "sink": sk,
            }
        )
    return in_maps


def run(inputs, trace=False):
    """Run the SPMD kernel; returns (full_output, exec_time_ns_or_None)."""
    nc = _get_program()
    in_maps = _host_prepare(inputs)
    res = run_bass_kernel_spmd(
        nc, in_maps, core_ids=list(range(N_CORES)), trace=trace
    )
    total = np.zeros((NQC, P, KC * QCH), dtype=np.float32)
    for c in range(N_CORES):
        total += np.asarray(res.results[c]["out"]).astype(np.float32)
    # [NQC, P, KC, QCH] -> [KC, P, NQC, QCH] -> [HID, S]
    full = (
        total.reshape(NQC, P, KC, QCH)
        .transpose(2, 1, 0, 3)
        .reshape(HID, S)
    )
    out = np.ascontiguousarray(full.T).reshape(B, S, HID).astype(np.float32)
    return out, res.exec_time_ns


def kernel(**inputs) -> np.ndarray:
    out, _ = run(inputs, trace=False)
    return out
